# revision 28
# baseline (speedup 1.0000x reference)
"""Luong 'general' attention scoring kernel for 8 TRN2 NeuronCores.

Reference computation:
    h   = decoder_hidden[0]            # [H]
    enc = encoder_outputs[:, 0, :]     # [S, H]
    scores = (enc @ W.T + b) @ h       # [S]
    attn   = softmax(scores)           # -> [1, 1, S]

Algebraic refactor used here (exact math):
    (enc @ W.T + b) @ h = enc @ (h @ W) + (b . h)
The (b . h) term shifts every score equally, and softmax is shift-invariant,
so b drops out entirely. That collapses the S*H*H matmul into a memory-bound
mat-vec: per core, stream the enc shard once and dot each row with v = h @ W.

Sharding: encoder_outputs split along seq_len across 8 cores (sequence
parallel); W and decoder_hidden replicated. Each core computes
    v = h @ W                  (TensorE, PSUM accumulate over 8 k-chunks)
    v broadcast to 128 partitions via ones[1,128] matmul (outer product)
    prod = enc_tile * v        (VectorE tensor_tensor, fp32)
    scores[:, t] = sum(prod)   (ScalarE activation Copy with accum_out)
    m_p = max_t scores[p,t]; p_pt = exp(scores - m_p); z_p = sum_t p_pt
(per-partition softmax stats) and returns [128, 34] = [p | m | z].
The host merges the 8x128 partial softmaxes in float64 (standard online
softmax combine) - a pure gather/rescale step on 32k values.
"""

import sys

for _p in ("/opt/trn_rl_repo",):
    if _p not in sys.path:
        sys.path.insert(0, _p)

import numpy as np

import concourse.bass as bass
import concourse.mybir as mybir
from concourse import bacc
from concourse.bass_utils import run_bass_kernel_spmd
from concourse.tile import TileContext

N_CORES = 8
SEQ = 32768
H = 1024
S_SHARD = SEQ // N_CORES  # 4096
P = 128
TILES = S_SHARD // P      # 32 score columns per core
GROUP = 2                 # s-tiles per DMA (1 MiB transfers)
GROUPS = TILES // GROUP   # 16
KCHUNKS = H // P          # 8

TRACE = False
LAST = {"exec_time_ns": None, "results": None}

_nc_cache = {}


def _build_nc():
    f32 = mybir.dt.float32
    nc = bacc.Bacc()

    enc = nc.dram_tensor("enc", [S_SHARD, H], f32, kind="ExternalInput")
    w = nc.dram_tensor("w", [H, H], f32, kind="ExternalInput")
    h = nc.dram_tensor("h", [1, H], f32, kind="ExternalInput")
    out = nc.dram_tensor("out", [P, TILES + 2], f32, kind="ExternalOutput")

    with TileContext(nc) as tc:
        with (
            tc.tile_pool(name="consts", bufs=1) as consts,
            tc.tile_pool(name="encp", bufs=12) as encp,
        ):
            # Pre-warm the exp activation table so the ~2.7us ACT_TABLE_LOAD
            # overlaps the enc streaming instead of landing on the tail.
            warm = consts.tile([1, 1], f32)
            nc.vector.memset(warm[:], 0.0)
            nc.scalar.activation(warm[:], warm[:], mybir.ActivationFunctionType.Exp)

            ones = consts.tile([1, P], f32)
            nc.vector.memset(ones[:], 1.0)

            # h as [128, 8]: h_sb[p, k] = h[0, k*128 + p]
            h_sb = consts.tile([P, KCHUNKS], f32)
            nc.sync.dma_start(out=h_sb[:], in_=h.rearrange("o (k p) -> p (o k)", p=P))

            # W as [128, 8, 1024]: w_sb[p, k, n] = W[k*128 + p, n].
            # One DMA per k-chunk so the v matmuls pipeline behind the
            # W stream instead of waiting for all 4 MiB.
            w_sb = consts.tile([P, KCHUNKS, H], f32)
            w_r = w.rearrange("(k p) n -> k p n", p=P)
            for k in range(KCHUNKS):
                nc.sync.dma_start(out=w_sb[:, k], in_=w_r[k])

            v_row = consts.tile([1, H], f32)
            v_rep = consts.tile([P, H], f32)

            with tc.tile_pool(name="vpsum", bufs=1, space="PSUM") as vpsum:
                # PE prelude: walrus allows only one semaphore wait on a
                # matmul's load-weights slot, so absorb each producer
                # semaphore into the PE vector clock one instruction at
                # a time.
                pe_warm1 = vpsum.tile([1, 1], f32, tag="warm1")
                nc.tensor.matmul(pe_warm1[:], ones[:, 0:1], ones[:, 0:1], start=True, stop=True)
                pe_warm2 = vpsum.tile([1, 1], f32, tag="warm2")
                nc.tensor.matmul(pe_warm2[:], h_sb[:, 0:1], h_sb[:, 0:1], start=True, stop=True)
                pe_warm3 = vpsum.tile([1, 1], f32, tag="warm3")
                nc.tensor.matmul(pe_warm3[:], w_sb[:, 0, 0:1], w_sb[:, 0, 0:1], start=True, stop=True)

                # v = h @ W : v[n] = sum_d h[d] W[d, n], accumulated over the
                # 8 k-chunks; k-outer so each chunk's matmuls start as soon
                # as its DMA lands.
                v_ps = vpsum.tile([1, H], f32)
                for k in range(KCHUNKS):
                    for n in range(2):
                        nc.tensor.matmul(
                            v_ps[:, n * 512 : (n + 1) * 512],
                            h_sb[:, k : k + 1],
                            w_sb[:, k, n * 512 : (n + 1) * 512],
                            start=(k == 0),
                            stop=(k == KCHUNKS - 1),
                        )

                # Broadcast v to all 128 partitions (outer product
                # ones^T x v), pipelined per 512-column half across
                # ACT (psum->sbuf), PE (broadcast matmul), DVE (psum->sbuf).
                for n in range(2):
                    sl = slice(n * 512, (n + 1) * 512)
                    nc.scalar.copy(v_row[:, sl], v_ps[:, sl])
                    v_bc_ps = vpsum.tile([P, 512], f32, tag="vbc")
                    nc.tensor.matmul(v_bc_ps[:], ones[:], v_row[:, sl], start=True, stop=True)
                    nc.vector.tensor_copy(v_rep[:, sl], v_bc_ps[:])

            scores = consts.tile([P, TILES], f32)
            outt = consts.tile([P, TILES + 2], f32)
            dump = consts.tile([P, H], f32)  # write-only ACT main output

            # enc[(g*GROUP + j)*128 + p, n] -> [g][p, j, n]
            enc_r = enc.rearrange("(g j p) n -> g p j n", p=P, j=GROUP)
            with tc.tile_pool(name="prodp", bufs=3, space="PSUM") as prodp:
                for g in range(GROUPS):
                    et = encp.tile([P, GROUP, H], f32, tag="enc")
                    nc.sync.dma_start(out=et[:], in_=enc_r[g])
                    for j in range(GROUP):
                        t = g * GROUP + j
                        prod = prodp.tile([P, H], f32, tag="prod")
                        nc.vector.tensor_tensor(
                            prod[:], et[:, j], v_rep[:], mybir.AluOpType.mult
                        )
                        if t in (10, 21):
                            # ACT paces the loop (~1.33us/tile vs DVE
                            # ~1.13); hand two reduces to DVE to balance.
                            nc.vector.tensor_reduce(
                                out=scores[:, t : t + 1],
                                in_=prod[:],
                                axis=mybir.AxisListType.X,
                                op=mybir.AluOpType.add,
                            )
                        else:
                            nc.scalar.activation(
                                dump[:],
                                prod[:],
                                mybir.ActivationFunctionType.Copy,
                                accum_out=scores[:, t : t + 1],
                            )

            # Per-partition softmax stats: m, exp(s - m), z.
            nc.vector.reduce_max(
                out=outt[:, TILES : TILES + 1], in_=scores[:], axis=mybir.AxisListType.X
            )
            neg_m = consts.tile([P, 1], f32)
            nc.vector.tensor_scalar_mul(neg_m[:], outt[:, TILES : TILES + 1], -1.0)
            nc.scalar.activation(
                outt[:, 0:TILES],
                scores[:],
                mybir.ActivationFunctionType.Exp,
                bias=neg_m[:],
                scale=1.0,
                accum_out=outt[:, TILES + 1 : TILES + 2],
            )
            nc.sync.dma_start(out=out[:, :], in_=outt[:])

    nc.compile()
    return nc


def kernel(decoder_hidden, encoder_outputs, W, b):
    if "nc" not in _nc_cache:
        _nc_cache["nc"] = _build_nc()
    nc = _nc_cache["nc"]

    enc = np.ascontiguousarray(
        np.asarray(encoder_outputs, dtype=np.float32).reshape(SEQ, H)
    )
    w = np.ascontiguousarray(np.asarray(W, dtype=np.float32))
    h = np.ascontiguousarray(np.asarray(decoder_hidden, dtype=np.float32).reshape(1, H))
    # b shifts every score by the same (b . h); softmax is shift-invariant,
    # so it cannot affect the output and is intentionally unused.

    in_maps = [
        {"enc": enc[i * S_SHARD : (i + 1) * S_SHARD], "w": w, "h": h}
        for i in range(N_CORES)
    ]
    res = run_bass_kernel_spmd(nc, in_maps, core_ids=list(range(N_CORES)), trace=TRACE)
    LAST["exec_time_ns"] = res.exec_time_ns
    LAST["results"] = res

    outs = np.stack([np.asarray(res.results[i]["out"]) for i in range(N_CORES)])
    ps = outs[:, :, 0:TILES].astype(np.float64)   # [8, 128, 32]
    ms = outs[:, :, TILES].astype(np.float64)     # [8, 128]
    zs = outs[:, :, TILES + 1].astype(np.float64) # [8, 128]

    m_global = ms.max()
    scale = np.exp(ms - m_global)                 # [8, 128]
    denom = float((zs * scale).sum())
    attn = ps * scale[:, :, None] / denom         # [8, 128, 32]
    # s = core*4096 + t*128 + p  ->  [core, t, p] order
    attn = attn.transpose(0, 2, 1).reshape(SEQ)
    return attn.astype(np.float32)[None, None, :]


# revision 30
# speedup vs baseline: 1.0170x; 1.0170x over previous
"""Luong 'general' attention scoring kernel for 8 TRN2 NeuronCores.

Reference computation:
    h   = decoder_hidden[0]            # [H]
    enc = encoder_outputs[:, 0, :]     # [S, H]
    scores = (enc @ W.T + b) @ h       # [S]
    attn   = softmax(scores)           # -> [1, 1, S]

Algebraic refactor used here (exact math):
    (enc @ W.T + b) @ h = enc @ (h @ W) + (b . h)
The (b . h) term shifts every score equally, and softmax is shift-invariant,
so b drops out entirely. That collapses the S*H*H matmul into a memory-bound
mat-vec: per core, stream the enc shard once and dot each row with v = h @ W.

Sharding: encoder_outputs split along seq_len across 8 cores (sequence
parallel); W and decoder_hidden replicated. Each core computes
    v = h @ W                  (TensorE, PSUM accumulate over 8 k-chunks)
    v broadcast to 128 partitions via ones[1,128] matmul (outer product)
    prod = enc_tile * v        (VectorE tensor_tensor, fp32)
    scores[:, t] = sum(prod)   (ScalarE activation Copy with accum_out)
    m_p = max_t scores[p,t]; p_pt = exp(scores - m_p); z_p = sum_t p_pt
(per-partition softmax stats) and returns [128, 34] = [p | m | z].
The host merges the 8x128 partial softmaxes in float64 (standard online
softmax combine) - a pure gather/rescale step on 32k values.
"""

import sys

for _p in ("/opt/trn_rl_repo",):
    if _p not in sys.path:
        sys.path.insert(0, _p)

import numpy as np

import concourse.bass as bass
import concourse.mybir as mybir
from concourse import bacc
from concourse.bass_utils import run_bass_kernel_spmd
from concourse.tile import TileContext

N_CORES = 8
SEQ = 32768
H = 1024
S_SHARD = SEQ // N_CORES  # 4096
P = 128
TILES = S_SHARD // P      # 32 score columns per core
GROUP = 2                 # s-tiles per DMA (1 MiB transfers)
GROUPS = TILES // GROUP   # 16
KCHUNKS = H // P          # 8

TRACE = False
LAST = {"exec_time_ns": None, "results": None}

_nc_cache = {}


def _build_nc():
    f32 = mybir.dt.float32
    nc = bacc.Bacc()

    enc = nc.dram_tensor("enc", [S_SHARD, H], f32, kind="ExternalInput")
    w = nc.dram_tensor("w", [H, H], f32, kind="ExternalInput")
    h = nc.dram_tensor("h", [1, H], f32, kind="ExternalInput")
    out = nc.dram_tensor("out", [P, TILES + 2], f32, kind="ExternalOutput")

    with TileContext(nc) as tc:
        with (
            tc.tile_pool(name="consts", bufs=1) as consts,
            tc.tile_pool(name="encp", bufs=12) as encp,
        ):
            # Pre-warm the exp activation table so the ~2.7us ACT_TABLE_LOAD
            # overlaps the enc streaming instead of landing on the tail.
            warm = consts.tile([1, 1], f32)
            nc.vector.memset(warm[:], 0.0)
            nc.scalar.activation(warm[:], warm[:], mybir.ActivationFunctionType.Exp)

            ones = consts.tile([1, P], f32)
            nc.vector.memset(ones[:], 1.0)

            # h as [128, 8]: h_sb[p, k] = h[0, k*128 + p]
            h_sb = consts.tile([P, KCHUNKS], f32)
            nc.sync.dma_start(out=h_sb[:], in_=h.rearrange("o (k p) -> p (o k)", p=P))

            # W as [128, 8, 1024]: w_sb[p, k, n] = W[k*128 + p, n].
            # One DMA per k-chunk so the v matmuls pipeline behind the
            # W stream instead of waiting for all 4 MiB.
            w_sb = consts.tile([P, KCHUNKS, H], f32)
            w_r = w.rearrange("(k p) n -> k p n", p=P)
            for k in range(KCHUNKS):
                nc.sync.dma_start(out=w_sb[:, k], in_=w_r[k])

            v_row = consts.tile([1, H], f32)
            v_rep = consts.tile([P, H], f32)

            with tc.tile_pool(name="vpsum", bufs=1, space="PSUM") as vpsum:
                # PE prelude: walrus allows only one semaphore wait on a
                # matmul's load-weights slot, so absorb each producer
                # semaphore into the PE vector clock one instruction at
                # a time.
                pe_warm1 = vpsum.tile([1, 1], f32, tag="warm1")
                nc.tensor.matmul(pe_warm1[:], ones[:, 0:1], ones[:, 0:1], start=True, stop=True)
                pe_warm2 = vpsum.tile([1, 1], f32, tag="warm2")
                nc.tensor.matmul(pe_warm2[:], h_sb[:, 0:1], h_sb[:, 0:1], start=True, stop=True)
                pe_warm3 = vpsum.tile([1, 1], f32, tag="warm3")
                nc.tensor.matmul(pe_warm3[:], w_sb[:, 0, 0:1], w_sb[:, 0, 0:1], start=True, stop=True)

                # v = h @ W : v[n] = sum_d h[d] W[d, n], accumulated over the
                # 8 k-chunks; k-outer so each chunk's matmuls start as soon
                # as its DMA lands.
                v_ps = vpsum.tile([1, H], f32)
                for k in range(KCHUNKS):
                    for n in range(2):
                        nc.tensor.matmul(
                            v_ps[:, n * 512 : (n + 1) * 512],
                            h_sb[:, k : k + 1],
                            w_sb[:, k, n * 512 : (n + 1) * 512],
                            start=(k == 0),
                            stop=(k == KCHUNKS - 1),
                        )

                # Broadcast v to all 128 partitions (outer product
                # ones^T x v), pipelined per 512-column half across
                # ACT (psum->sbuf), PE (broadcast matmul), DVE (psum->sbuf).
                for n in range(2):
                    sl = slice(n * 512, (n + 1) * 512)
                    nc.scalar.copy(v_row[:, sl], v_ps[:, sl])
                    v_bc_ps = vpsum.tile([P, 512], f32, tag="vbc")
                    nc.tensor.matmul(v_bc_ps[:], ones[:], v_row[:, sl], start=True, stop=True)
                    nc.vector.tensor_copy(v_rep[:, sl], v_bc_ps[:])

            outt = consts.tile([P, TILES + 2], f32)
            dump = consts.tile([P, H], f32)  # write-only ACT main output

            # enc[(g*GROUP + j)*128 + p, n] -> [g][p, j, n]
            enc_r = enc.rearrange("(g j p) n -> g p j n", p=P, j=GROUP)
            with (
                tc.tile_pool(name="prodp", bufs=3, space="PSUM") as prodp,
                tc.tile_pool(name="scorep", bufs=1, space="PSUM") as scorep,
            ):
                scores = scorep.tile([P, TILES], f32)
                for g in range(GROUPS):
                    et = encp.tile([P, GROUP, H], f32, tag="enc")
                    nc.sync.dma_start(out=et[:], in_=enc_r[g])
                    for j in range(GROUP):
                        t = g * GROUP + j
                        prod = prodp.tile([P, H], f32, tag="prod")
                        nc.vector.tensor_tensor(
                            prod[:], et[:, j], v_rep[:], mybir.AluOpType.mult
                        )
                        if t in (10, 21):
                            # ACT paces the loop (~1.33us/tile vs DVE
                            # ~1.13); hand two reduces to DVE to balance.
                            nc.vector.tensor_reduce(
                                out=scores[:, t : t + 1],
                                in_=prod[:],
                                axis=mybir.AxisListType.X,
                                op=mybir.AluOpType.add,
                            )
                        else:
                            nc.scalar.activation(
                                dump[:],
                                prod[:],
                                mybir.ActivationFunctionType.Copy,
                                accum_out=scores[:, t : t + 1],
                            )

                # Per-partition softmax stats: m, exp(s - m), z.
                nc.vector.reduce_max(
                    out=outt[:, TILES : TILES + 1], in_=scores[:], axis=mybir.AxisListType.X
                )
                neg_m = consts.tile([P, 1], f32)
                nc.vector.tensor_scalar_mul(neg_m[:], outt[:, TILES : TILES + 1], -1.0)
                nc.scalar.activation(
                    outt[:, 0:TILES],
                    scores[:],
                    mybir.ActivationFunctionType.Exp,
                    bias=neg_m[:],
                    scale=1.0,
                    accum_out=outt[:, TILES + 1 : TILES + 2],
                )
                nc.sync.dma_start(out=out[:, :], in_=outt[:])

    nc.compile()
    return nc


def kernel(decoder_hidden, encoder_outputs, W, b):
    if "nc" not in _nc_cache:
        _nc_cache["nc"] = _build_nc()
    nc = _nc_cache["nc"]

    enc = np.ascontiguousarray(
        np.asarray(encoder_outputs, dtype=np.float32).reshape(SEQ, H)
    )
    w = np.ascontiguousarray(np.asarray(W, dtype=np.float32))
    h = np.ascontiguousarray(np.asarray(decoder_hidden, dtype=np.float32).reshape(1, H))
    # b shifts every score by the same (b . h); softmax is shift-invariant,
    # so it cannot affect the output and is intentionally unused.

    in_maps = [
        {"enc": enc[i * S_SHARD : (i + 1) * S_SHARD], "w": w, "h": h}
        for i in range(N_CORES)
    ]
    res = run_bass_kernel_spmd(nc, in_maps, core_ids=list(range(N_CORES)), trace=TRACE)
    LAST["exec_time_ns"] = res.exec_time_ns
    LAST["results"] = res

    outs = np.stack([np.asarray(res.results[i]["out"]) for i in range(N_CORES)])
    ps = outs[:, :, 0:TILES].astype(np.float64)   # [8, 128, 32]
    ms = outs[:, :, TILES].astype(np.float64)     # [8, 128]
    zs = outs[:, :, TILES + 1].astype(np.float64) # [8, 128]

    m_global = ms.max()
    scale = np.exp(ms - m_global)                 # [8, 128]
    denom = float((zs * scale).sum())
    attn = ps * scale[:, :, None] / denom         # [8, 128, 32]
    # s = core*4096 + t*128 + p  ->  [core, t, p] order
    attn = attn.transpose(0, 2, 1).reshape(SEQ)
    return attn.astype(np.float32)[None, None, :]
